# revision 50
# baseline (speedup 1.0000x reference)
"""AttnLSTMDecoder Trainium2 kernel: builder + host preprocessing.

Sharding: encoder length axis L split evenly across 8 cores; per-step
softmax normalizer + context partials all-reduced via remote SBUF DMA
broadcast (mesh all-to-all, one hop). LSTM replicated on every core.
"""
import sys
sys.path.insert(0, '/opt/trn_rl_repo')
import numpy as np
import ml_dtypes
from contextlib import ExitStack
from concourse import bass, bacc, tile
mybir = bass.mybir

F32 = mybir.dt.float32
BF16 = mybir.dt.bfloat16
F8 = mybir.dt.float8e4
DR = mybir.MatmulPerfMode.DoubleRow
Tanh = mybir.ActivationFunctionType.Tanh
Exp = mybir.ActivationFunctionType.Exp
ADD = mybir.AluOpType.add
MULT = mybir.AluOpType.mult

STATE = 100
ATT = 100
EMB = 100
VOCAB = 128
NCORES = 8


def build_kernel(Lc, T, n_tanh_chunks=4, ctx_groups=4, repeats=1, ablate_exchange=False, skip=(), wbufs=2, exchange="collective", fp8ctx=True):
    """Build the per-core SPMD kernel. Lc = L/8 (multiple of 512).

    exchange: "rdma" (SBUF-to-SBUF remote DMA all-gather), "collective"
    (ncfw AllGather via HBM bounce), or "none" (ablation; wrong result).
    """
    NCH = Lc // 128          # l-chunks of 128
    assert Lc % 512 == 0
    assert NCH % n_tanh_chunks == 0
    assert NCH % ctx_groups == 0 or True
    if ablate_exchange:
        exchange = "none"
    if fp8ctx:
        assert exchange == "collective"
    nc = bacc.Bacc("TRN2", target_bir_lowering=False, debug=False,
                   num_devices=NCORES)

    # ---------------- DRAM parameters (per-core) ----------------
    d_imB = nc.declare_dram_parameter("imB", [200, Lc], F32, isOutput=False)
    d_imTa = nc.declare_dram_parameter("imTa",
                                       [128, NCH * (224 if fp8ctx else 208)],
                                       F8 if fp8ctx else BF16, isOutput=False)
    d_w1T = nc.declare_dram_parameter("w1T", [200, ATT], F32, isOutput=False)
    d_vb = nc.declare_dram_parameter("vb", [ATT, 1], BF16, isOutput=False)
    d_Wg = nc.declare_dram_parameter("Wg", [401, 400], F32, isOutput=False)
    d_w2T = nc.declare_dram_parameter("w2T", [200, ATT], F32, isOutput=False)
    d_linTb = nc.declare_dram_parameter("linTb", [101, VOCAB], F32, isOutput=False)
    d_linsel = nc.declare_dram_parameter("linsel", [101, T], F32, isOutput=False)
    d_gemb = nc.declare_dram_parameter("gemb", [100, 4 * T], F32, isOutput=False)
    d_h0 = nc.declare_dram_parameter("h0aug", [101, 1], F32, isOutput=False)
    d_c0 = nc.declare_dram_parameter("c0", [STATE, 1], F32, isOutput=False)
    d_id = nc.declare_dram_parameter("id100", [100, 100], F32, isOutput=False)
    d_S = nc.declare_dram_parameter("Sout", [1, T], F32, isOutput=True)
    d_sel = nc.declare_dram_parameter("selout", [1, T], F32, isOutput=True)
    d_selEO = nc.declare_dram_parameter("selEO", [16, 2], F32, isOutput=False)
    cc_shape = ([1, 208], [8, 208]) if fp8ctx else ([2, 416], [16, 416])
    ccin = [nc.dram_tensor(f"ccin{i}", cc_shape[0], F32) for i in range(2)]
    ccout = [nc.dram_tensor(f"ccout{i}", cc_shape[1], F32, addr_space="Shared")
             for i in range(2)]
    if exchange == "rdma":
        # per-sender-slot remote sems, double-buffered by step parity (the
        # k+2-vs-k chain is provably ordered; k+1-vs-k is not) + local
        # send-completion sem; cleared at entry before the barrier
        rsems = [[nc.alloc_semaphore(f"rsem{p}_{d}") for d in range(1, NCORES)]
                 for p in range(2)]
        lsem = nc.alloc_semaphore("lsem")
        ccbar_in = nc.dram_tensor("ccbar_in", [1, 1], mybir.dt.uint8)
        ccbar_out = nc.dram_tensor("ccbar_out", [NCORES, 1], mybir.dt.uint8,
                                   addr_space="Shared")

    ext_waits = []   # (BassInstruction, sem, value) attached post-scheduling
    with tile.TileContext(nc) as tc, ExitStack() as ctxs:
        # pools
        P = ctxs.enter_context(tc.tile_pool(name="static", bufs=1))
        W = ctxs.enter_context(tc.tile_pool(name="work", bufs=wbufs))
        PS = ctxs.enter_context(tc.tile_pool(name="psum", bufs=1,
                                             space="PSUM"))

        # ---------------- static SBUF tiles ----------------
        imB1 = P.tile([100, Lc], F32, tag="imB1")
        imB2 = P.tile([100, Lc], F32, tag="imB2")
        imTa = P.tile([128, NCH * (224 if fp8ctx else 208)],
                      F8 if fp8ctx else BF16, tag="imTa")
        sel8 = P.tile([8, 1], F32, tag="sel8")
        w1Ta = P.tile([100, ATT], F32, tag="w1Ta")
        w1Tb = P.tile([100, ATT], F32, tag="w1Tb")
        vb = P.tile([ATT, 1], BF16, tag="vb")
        Wg_ctx1 = P.tile([128, 400], F32, tag="Wgc1")
        Wg_ctx2 = P.tile([72, 400], F32, tag="Wgc2")
        Wg_h = P.tile([100, 400], F32, tag="Wgh")
        gemb = P.tile([100, 4 * T], F32, tag="gemb")
        id100 = P.tile([100, 100], F32, tag="id100")
        w2Th = P.tile([100, ATT], F32, tag="w2Th")
        w2Tc = P.tile([100, ATT], F32, tag="w2Tc")
        linTb = P.tile([101, VOCAB], F32, tag="linTb")
        linsel = P.tile([101, T], F32, tag="linsel")
        h_aug = P.tile([101, 1], F32, tag="haug")
        c_sb = P.tile([STATE, 1], F32, tag="c")
        w1tb = P.tile([ATT, Lc], BF16, tag="w1tb")
        tanh_sb = P.tile([ATT, Lc], BF16, tag="tanhsb")
        Sbuf = P.tile([1, T], F32, tag="Sbuf")
        selbuf = P.tile([1, T], F32, tag="selbuf")
        ones1 = P.tile([1, 1], F32, tag="ones1")
        ones128 = P.tile([1, 128], F32, tag="ones128")
        selEO = P.tile([16, 2], F32, tag="selEO")
        if exchange == "rdma":
            # ping-pong send payload + gathered slots: cols 0=ctx_a(128),
            # 1=den, 2=ctx_b(rows 0:72); 8 f32 per slot for 32B transfers
            flat = [P.tile([128, 8], F32, tag=f"flat{i}", name=f"flat{i}")
                    for i in range(2)]
            gf = [P.tile([128, 8 * NCORES], F32, tag=f"gf{i}", name=f"gf{i}")
                  for i in range(2)]

        # ---------------- init ----------------
        nc.sync.dma_start(imB1[:], d_imB[0:100, :])
        nc.sync.dma_start(imB2[:], d_imB[100:200, :])
        nc.sync.dma_start(imTa[:], d_imTa[:])
        nc.sync.dma_start(w1Ta[:], d_w1T[0:100, :])
        nc.sync.dma_start(w1Tb[:], d_w1T[100:200, :])
        nc.sync.dma_start(vb[:], d_vb[:])
        nc.sync.dma_start(Wg_ctx1[:], d_Wg[0:128, :])
        nc.sync.dma_start(Wg_ctx2[:], d_Wg[128:200, :])
        nc.sync.dma_start(Wg_h[:], d_Wg[301:401, :])
        nc.sync.dma_start(gemb[:], d_gemb[:])
        nc.sync.dma_start(id100[:], d_id[:])
        nc.sync.dma_start(w2Th[:], d_w2T[0:100, :])
        nc.sync.dma_start(w2Tc[:], d_w2T[100:200, :])
        nc.sync.dma_start(linTb[:], d_linTb[:])
        nc.sync.dma_start(linsel[:], d_linsel[:])
        nc.sync.dma_start(h_aug[:], d_h0[:])
        nc.sync.dma_start(c_sb[:], d_c0[:])
        nc.gpsimd.memset(ones1[:], 1.0)
        nc.gpsimd.memset(ones128[:], 1.0)
        nc.gpsimd.memset(sel8[:], 1.0)
        nc.sync.dma_start(selEO[:], d_selEO[:])
        if exchange == "rdma":
            # no gf memset: every slot byte is remotely/self written each step
            # before the tree reads it, and a local init write would look like
            # a cross-core race to the detector (collectives carry no
            # happens-before watermarks)
            for i in range(2):
                nc.gpsimd.memset(flat[i][:], 0.0)
            # clear exchange sems, THEN barrier: a peer's first send can only
            # follow its own barrier-completion, which needs our arrival,
            # which follows our clears — so no increment is ever lost
            for s in rsems[0] + rsems[1] + [lsem]:
                nc.gpsimd.sem_clear(s)
            nc.gpsimd.collective_compute(
                "AllGather", mybir.AluOpType.bypass,
                replica_groups=[list(range(NCORES))],
                ins=[ccbar_in.ap().opt()], outs=[ccbar_out.ap().opt()])

            def issue_preps(parity):
                # stage the 7 send-descriptor frames for the NEXT trigger;
                # desc-gen runs on the Pool Q7 during the tanh phase, and the
                # payload read happens at trigger time
                for d in range(1, NCORES):
                    rdest = [None] * NCORES
                    rdest[d] = (0, d)
                    nc.gpsimd.remote_dma_broadcast(
                        gf[parity][:, 8 * d:8 * d + 8], flat[parity][:, 0:8],
                        rsems[parity][d - 1], lsem, rdests=rdest)
            issue_preps(0)

        # w1t = w1 @ input_mat   -> [ATT, Lc] bf16
        for j in range(Lc // 512):
            w1p = PS.tile([ATT, 512], F32, tag="w1p")
            sl = slice(512 * j, 512 * (j + 1))
            nc.tensor.matmul(w1p[:], w1Ta[:], imB1[:, sl], start=True, stop=False)
            nc.tensor.matmul(w1p[:], w1Tb[:], imB2[:, sl], start=False, stop=True)
            nc.scalar.copy(w1tb[:, sl], w1p[:])

        CH = NCH // n_tanh_chunks  # l-chunks per tanh chunk
        # tapered chunk bounds: a small final chunk shortens the PE trail
        # (scores+ctx of the last chunk run after the last tanh finishes)
        if n_tanh_chunks == 4 and NCH % 16 == 0 and NCH >= 32:
            tail = NCH // 16
            big = (NCH - tail + 2) // 3
            bounds = [0, big, 2 * big, NCH - tail, NCH]
        else:
            bounds = [i * CH for i in range(n_tanh_chunks + 1)]

        # ---------------- decode steps ----------------
        def emit_logits(tt):
            lg_full = PS.tile([1, 512], F32, tag="lg")
            lg_ps = lg_full[:, 0:129]
            nc.tensor.matmul(lg_ps[0:1, 0:128], h_aug[:, 0:1], linTb[:],
                             start=True, stop=True)
            nc.tensor.matmul(lg_ps[0:1, 128:129], h_aug[:, 0:1],
                             linsel[:, tt:tt + 1], start=True, stop=True)
            exps = W.tile([1, VOCAB], F32, tag="exps")
            nc.scalar.activation(exps[:], lg_ps[0:1, 0:128], Exp,
                                 accum_out=Sbuf[0:1, tt:tt + 1])
            nc.vector.tensor_copy(selbuf[0:1, tt:tt + 1],
                                  lg_ps[0:1, 128:129])

        steps = [tt for _ in range(repeats) for tt in range(T)]
        for k, t in enumerate(steps):
            # w2dt = w2 @ [h; c]  -> bias for tanh
            w2p_full = PS.tile([ATT, 512], F32, tag="w2p")
            w2p = w2p_full[:, 0:1]
            nc.tensor.matmul(w2p[:], w2Th[:], h_aug[0:100, 0:1], start=True, stop=False)
            nc.tensor.matmul(w2p[:], w2Tc[:], c_sb[:], start=False, stop=True)
            bias_sb = W.tile([ATT, 1], F32, tag="bias")
            nc.scalar.copy(bias_sb[:], w2p[:])

            # gates from h + host-precomputed emb/bias part (via identity
            # matmul) — known at step start; issue early so the PE covers
            # them while ACT runs the first tanh chunk
            gates_full = PS.tile([100, 512], F32, tag="gates")
            gates_ps = gates_full[:, 0:4]
            nc.tensor.matmul(gates_ps[:], id100[:],
                             gemb[:, 4 * t:4 * t + 4], start=True, stop=False,
                             skip_group_check=True)
            for g in range(4 if "gates" not in skip else 0):
                gs = slice(100 * g, 100 * (g + 1))
                nc.tensor.matmul(gates_ps[:, g:g + 1], Wg_h[:, gs],
                                 h_aug[0:100, 0:1], start=False, stop=False,
                                 skip_group_check=True)

            if exchange == "rdma":
                gfb, flb = gf[k % 2], flat[k % 2]

            scores_full = PS.tile([128, 512], F32, tag="scores")
            scores_ps = scores_full[:, 0:NCH]
            if fp8ctx:
                # fp8 att at 16B column stride (DoubleRow weight constraint)
                att_sb = W.tile([128, NCH * 16], F8, tag="att")
                att3 = att_sb[:].rearrange("p (n s) -> p n s", s=16)
                imTa3 = imTa[:].rearrange("p (k n) -> p k n", n=224)
            else:
                att_sb = W.tile([128, NCH], BF16, tag="att")
            ctx_full = PS.tile([2, 512], F32, tag="ctx")
            ctx_ps = ctx_full[0:1, 0:208] if fp8ctx else ctx_full[:, 0:416]

            # software-pipelined emission: l-chunks in groups of GSZ; ctx of
            # group g is emitted after scores of group g+1 so the PE's
            # weight-load path (score LDWs) overlaps its streaming path
            # (ctx matmuls) via the LDW pull-ahead window, and exp runs at
            # group granularity on ACT between tanh chunks.
            GSZ = 8
            NG = NCH // GSZ              # 8 groups
            TCH = 2                      # groups per tanh chunk

            def emit_tanh(ti):
                lo, hi = ti * TCH * GSZ * 128, (ti + 1) * TCH * GSZ * 128
                nc.scalar.activation(tanh_sb[:, lo:hi], w1tb[:, lo:hi],
                                     Tanh, bias=bias_sb[:, 0:1])

            def emit_scores(g):
                for c in range(g * GSZ, (g + 1) * GSZ):
                    nc.tensor.matmul(scores_ps[:, c:c + 1],
                                     tanh_sb[:, c * 128:(c + 1) * 128],
                                     vb[:], start=True, stop=True)

            def emit_exp(g):
                lo, hi = g * GSZ, (g + 1) * GSZ
                if fp8ctx:
                    nc.scalar.activation(att3[:, lo:hi, 0:1],
                                         scores_ps[:, lo:hi], Exp)
                else:
                    nc.scalar.activation(att_sb[:, lo:hi],
                                         scores_ps[:, lo:hi], Exp)

            def emit_ctx(g):
                for c in range(g * GSZ, (g + 1) * GSZ, 2):
                    if fp8ctx:
                        mv = imTa3[:, c:c + 2, 0:208]
                        wv = att_sb[:, 16 * c:16 * c + 32].rearrange(
                            "p (k s) -> p k s", k=2)[:, :, 0:1]
                        nc.tensor.matmul(ctx_ps[:],
                                         wv,
                                         mv,
                                         start=(c == 0), stop=(c >= NCH - 2),
                                         perf_mode=DR, skip_group_check=True)
                    else:
                        nc.tensor.matmul(ctx_ps[:],
                                         att_sb[:, c:c + 2],
                                         imTa[:, c * 208:(c + 2) * 208],
                                         start=(c == 0), stop=(c >= NCH - 2),
                                         skip_group_check=True)

            emit_tanh(0)
            emit_tanh(1)
            emit_scores(0)
            emit_exp(0)
            for g in range(1, NG):
                if g % TCH == 0 and g // TCH + 1 < NG // TCH:
                    emit_tanh(g // TCH + 1)
                emit_scores(g)
                emit_exp(g)
                emit_ctx(g - 1)
            emit_ctx(NG - 1)

            # partial (den|ctx) rows leave PSUM uncombined
            num_sb = W.tile([1, 208] if fp8ctx else [2, 416], F32, tag="num")
            if "combine" not in skip:
                nc.vector.tensor_copy(num_sb[:], ctx_ps[:])

            rd = W.tile([128, 1], F32, tag="rd")
            ctx_sb = W.tile([128, 2], F32, tag="ctxs")
            Copy = mybir.ActivationFunctionType.Copy

            if exchange == "rdma":
                # local combine: transpose own partials to partition columns
                # cols 0=ctx_a(128), 1=den bcast, 2=ctx_b(0:72); the 0/1
                # selector columns pick the valid half-row of each segment
                sE, sO = selEO[0:2, 0:1], selEO[0:2, 1:2]
                cu_full = PS.tile([128, 512], F32, tag="cu")
                nc.tensor.matmul(cu_full[:, 0:1], num_sb[:, 1:129], sE,
                                 start=True, stop=False)
                nc.tensor.matmul(cu_full[:, 0:1], num_sb[:, 209:337], sO,
                                 start=False, stop=True)
                nc.tensor.matmul(cu_full[:, 1:2],
                                 num_sb[:, 0:1].to_broadcast((2, 128)), sE,
                                 start=True, stop=False)
                nc.tensor.matmul(cu_full[:, 1:2],
                                 num_sb[:, 208:209].to_broadcast((2, 128)), sO,
                                 start=False, stop=True)
                nc.tensor.matmul(cu_full[0:72, 2:3], num_sb[:, 129:201], sE,
                                 start=True, stop=False)
                nc.tensor.matmul(cu_full[0:72, 2:3], num_sb[:, 337:409], sO,
                                 start=False, stop=True)
                # stage the send payload (don't overwrite until the sends that
                # last used this buffer have drained; the wait is attached
                # post-scheduling so the single-core tile pass can't deadlock)
                inst = nc.vector.tensor_copy(flb[:, 0:2], cu_full[:, 0:2])
                if k >= 2:
                    # all sends through step k-1 drained (per-step completions
                    # interleave across lanes, so only full-prefix counts are
                    # provable thresholds); covers this buffer's k-2 sends
                    ext_waits.append((inst, lsem, 112 * k))
                nc.vector.tensor_copy(flb[0:72, 2:3], cu_full[0:72, 2:3])
                nc.vector.tensor_copy(gfb[:, 0:8], flb[:, 0:8])  # self slot
                # fire the frames staged last step; the declared write of the
                # payload region (WAW vs the copies) makes tile order the
                # trigger after them and prove it with an engine sem
                nc.gpsimd.trigger_dma(count=None,
                                      signals_writable=[flb[:, 0:3]])
                if k + 1 < len(steps):
                    issue_preps((k + 1) % 2)
                if "post" in skip:
                    continue
                # wait for all 7 peers' step-k payloads, then tree-reduce the
                # 8 slots in place; col 0=ctx_a, 1=den, 2=ctx_b
                inst = nc.vector.tensor_tensor(gfb[:, 0:32], gfb[:, 0:32],
                                               gfb[:, 32:64], op=ADD)
                for d in range(1, NCORES):
                    ext_waits.append((inst, rsems[k % 2][d - 1],
                                      2 * (k // 2 + 1)))
                nc.vector.tensor_tensor(gfb[:, 0:16], gfb[:, 0:16],
                                        gfb[:, 16:32], op=ADD)
                nc.vector.tensor_tensor(gfb[:, 0:8], gfb[:, 0:8],
                                        gfb[:, 8:16], op=ADD)
                nc.vector.reciprocal(rd[:], gfb[:, 1:2])
                nc.scalar.activation(ctx_sb[:, 0:1], gfb[:, 0:1], Copy,
                                     scale=rd[:, 0:1])
                nc.scalar.activation(ctx_sb[0:72, 1:2], gfb[0:72, 2:3], Copy,
                                     scale=rd[0:72, 0:1])
            else:
                # ---- exchange: AllGather the partial rows ----
                gather = W.tile([8, 208] if fp8ctx else [16, 416], F32,
                                tag="gather")
                if exchange == "none":
                    nc.vector.tensor_copy(gather[0:2, :], num_sb[:])
                else:
                    cin, cout = ccin[t % 2], ccout[t % 2]
                    nc.sync.dma_start(cin[:], num_sb[:])
                    nc.gpsimd.collective_compute(
                        "AllGather", mybir.AluOpType.bypass,
                        replica_groups=[list(range(NCORES))],
                        ins=[cin.ap().opt()], outs=[cout.ap().opt()])
                    # previous step's logits fill the collective dead window
                    if k > 0:
                        emit_logits(steps[k - 1])
                    nc.sync.dma_start(gather[:], cout[:])

                if "post" in skip:
                    continue
                if fp8ctx:
                    # reduce the 8 gathered [1,208] rows AND transpose to
                    # partition columns in 3 matmuls with a ones selector
                    cu_full = PS.tile([128, 512], F32, tag="cu")
                    g_ = gather[0:8, :]
                    nc.tensor.matmul(cu_full[:, 0:1], g_[:, 1:129], sel8[:],
                                     start=True, stop=True)
                    nc.tensor.matmul(cu_full[0:72, 1:2], g_[:, 129:201],
                                     sel8[:], start=True, stop=True)
                    nc.tensor.matmul(cu_full[:, 2:3],
                                     g_[:, 0:1].to_broadcast((8, 128)),
                                     sel8[:], start=True, stop=True)
                else:
                    # reduce over ranks AND transpose to columns: even rows
                    # carry cols 0:208, odd rows cols 208:416; 0/1 masks
                    # select them into the same PSUM columns
                    KR = 2 if exchange == "none" else 16
                    sE, sO = selEO[0:KR, 0:1], selEO[0:KR, 1:2]
                    cu_full = PS.tile([128, 512], F32, tag="cu")
                    g_ = gather[0:KR, :]
                    nc.tensor.matmul(cu_full[:, 0:1], g_[:, 1:129], sE,
                                     start=True, stop=False)
                    nc.tensor.matmul(cu_full[:, 0:1], g_[:, 209:337], sO,
                                     start=False, stop=True)
                    nc.tensor.matmul(cu_full[0:72, 1:2], g_[:, 129:201], sE,
                                     start=True, stop=False)
                    nc.tensor.matmul(cu_full[0:72, 1:2], g_[:, 337:409], sO,
                                     start=False, stop=True)
                    # den reduced AND broadcast to all 128 partitions in one
                    # matmul (stationary free-dim stride 0 replicates it)
                    nc.tensor.matmul(cu_full[:, 2:3],
                                     g_[:, 0:1].to_broadcast((KR, 128)), sE,
                                     start=True, stop=False)
                    nc.tensor.matmul(cu_full[:, 2:3],
                                     g_[:, 208:209].to_broadcast((KR, 128)),
                                     sO, start=False, stop=True)
                nc.vector.reciprocal(rd[:], cu_full[:, 2:3])
                nc.scalar.activation(ctx_sb[:, 0:1], cu_full[:, 0:1], Copy,
                                     scale=rd[:, 0:1])
                nc.scalar.activation(ctx_sb[0:72, 1:2], cu_full[0:72, 1:2], Copy,
                                     scale=rd[0:72, 0:1])

            # close the gates accumulation with the ctx contributions
            for g in range(4 if "gates" not in skip else 0):
                gs = slice(100 * g, 100 * (g + 1))
                nc.tensor.matmul(gates_ps[:, g:g + 1], Wg_ctx1[:, gs],
                                 ctx_sb[:, 0:1], start=False, stop=False,
                                 skip_group_check=True)
                nc.tensor.matmul(gates_ps[:, g:g + 1], Wg_ctx2[:, gs],
                                 ctx_sb[0:72, 1:2], start=False, stop=True,
                                 skip_group_check=True)

            # LSTM elementwise
            t_all = W.tile([100, 4], F32, tag="tall")
            if "lstm" in skip:
                continue
            nc.scalar.activation(t_all[:], gates_ps[:], Tanh)
            sig = W.tile([100, 3], F32, tag="sig")
            nc.vector.tensor_scalar(sig[:], t_all[:, 0:3], 1.0, 0.5, ADD, MULT)
            tmp1 = W.tile([100, 1], F32, tag="tmp1")
            tmp2 = W.tile([100, 1], F32, tag="tmp2")
            nc.vector.tensor_tensor(tmp1[:], sig[:, 1:2], c_sb[:], op=MULT)
            nc.vector.tensor_tensor(tmp2[:], sig[:, 0:1], t_all[:, 3:4],
                                    op=MULT)
            nc.vector.tensor_tensor(c_sb[:], tmp1[:], tmp2[:], op=ADD)
            tanh_c = W.tile([100, 1], F32, tag="tanhc")
            nc.scalar.activation(tanh_c[:], c_sb[:], Tanh)
            nc.vector.tensor_tensor(h_aug[0:100, 0:1], sig[:, 2:3], tanh_c[:],
                                    op=MULT)
            if exchange != "collective" and "logits" not in skip:
                emit_logits(t)

        if exchange == "collective":
            emit_logits(steps[-1])
        nc.sync.dma_start(d_S[:], Sbuf[:])
        nc.sync.dma_start(d_sel[:], selbuf[:])

    # cross-core sem waits are invisible to the single-core tile scheduler
    # (it would deadlock); attach them to the scheduled instructions now —
    # compile()'s generate_event_semaphores legalizes multi-wait instructions
    for inst, sem, val in ext_waits:
        inst.wait_op(sem, val, "sem-ge", check=False)

    return nc


# =================== host preprocessing ===================

def _lstm_step_np(x, h, c, W_ih, W_hh, b_ih, b_hh):
    gates = W_ih @ x + b_ih + W_hh @ h + b_hh
    i, f, g, o = np.split(gates, 4)
    sig = lambda v: 1.0 / (1.0 + np.exp(-v))
    c = sig(f) * c + sig(i) * np.tanh(g)
    h = sig(o) * np.tanh(c)
    return h, c


def prep_inputs(inputs, Lc, T):
    """Produce the 8 per-core in_maps from the full problem inputs."""
    im = np.asarray(inputs["input_mat"], np.float32)        # [200, L]
    output_ids = np.asarray(inputs["output_ids"]).astype(np.int64)
    W_ih = np.asarray(inputs["W_ih"], np.float32)
    W_hh = np.asarray(inputs["W_hh"], np.float32)
    b_ih = np.asarray(inputs["b_ih"], np.float32)
    b_hh = np.asarray(inputs["b_hh"], np.float32)
    w1 = np.asarray(inputs["w1"], np.float32)
    w2 = np.asarray(inputs["w2"], np.float32)
    v_w = np.asarray(inputs["v_w"], np.float32)
    lin_w = np.asarray(inputs["lin_w"], np.float32)
    lin_b = np.asarray(inputs["lin_b"], np.float32)
    emb = np.asarray(inputs["emb"], np.float32)
    eos = int(np.asarray(inputs["eos_id"]))

    L = im.shape[1]
    assert Lc * NCORES == L and len(output_ids) == T
    NCH = Lc // 128

    # priming LSTM step on host (exact fp32 math, tiny)
    x0 = np.concatenate([np.zeros(200, np.float32), emb[eos]])
    h0, c0 = _lstm_step_np(x0, np.zeros(100, np.float32),
                           np.zeros(100, np.float32), W_ih, W_hh, b_ih, b_hh)
    h0aug = np.concatenate([h0, [1.0]]).astype(np.float32).reshape(101, 1)

    # gates weights: reorder [i,f,g,o] -> [i,f,o,g], scale i,f,o by 0.5,
    # columns [ctx(200); emb(100); bias(1); h(100)]; transposed for lhsT.
    Wcomb = np.concatenate([W_ih, W_hh], axis=1)            # [400, 400]
    bias = (b_ih + b_hh).astype(np.float32)                 # [400]
    order = np.concatenate([np.arange(100), np.arange(100, 200),
                            np.arange(300, 400), np.arange(200, 300)])
    Wr = Wcomb[order]                                       # rows i,f,o,g
    br = bias[order].copy()
    scale = np.ones((400, 1), np.float32); scale[0:300] = 0.5
    Wr = Wr * scale; br = br * scale[:, 0]
    Wg = np.zeros((401, 400), np.float32)
    # Wcomb columns = [x(300) ; h(100)] where x = [ctx(200); emb(100)]
    Wg[0:200] = Wr[:, 0:200].T       # ctx
    Wg[200:300] = Wr[:, 200:300].T   # emb
    Wg[300] = br                     # bias row
    Wg[301:401] = Wr[:, 300:400].T   # h
    # emb sequence (last_emb per step) with bias-1 row
    emb_seq = np.empty((T, EMB), np.float32)
    emb_seq[0] = emb[eos]
    emb_seq[1:] = emb[output_ids[:T - 1]]
    embaug = np.concatenate([emb_seq.T, np.ones((1, T), np.float32)], axis=0)
    # host-precomputed per-step emb+bias gate contributions:
    # gemb[p, 4t+g] = (Wg[200:301].T @ embaug)[100g+p, t]
    GE = Wg[200:301].T @ embaug                              # [400, T]
    gemb = GE.reshape(4, 100, T).transpose(1, 2, 0).reshape(100, 4 * T)
    gemb = np.ascontiguousarray(gemb, np.float32)
    # logits weights
    linTb = np.concatenate([lin_w.T, lin_b.reshape(1, -1)], axis=0)  # [101,128]
    linsel = np.concatenate([lin_w[output_ids[:T]].T,
                             lin_b[output_ids[:T]].reshape(1, -1)], axis=0)

    w1T = w1.T.copy()                 # [200, 100]
    w2T = w2.T.copy()                 # [200, 100]
    vbf = v_w.reshape(ATT, 1).astype(ml_dtypes.bfloat16)

    in_maps = []
    for cidx in range(NCORES):
        sl = slice(cidx * Lc, (cidx + 1) * Lc)
        imc = im[:, sl]                                    # [200, Lc]
        # imTa[q, c*201 + s] = im[s, c*128+q]; col 200 = 1.0
        blocks = imc.T.reshape(NCH, 128, 200)              # [c, q, s]
        PADW = 23 if FP8CTX else 7
        A = np.concatenate(
            [np.ones((NCH, 128, 1), np.float32), blocks,
             np.zeros((NCH, 128, PADW), np.float32)], axis=2)
        imTa = A.transpose(1, 0, 2).reshape(128, NCH * (201 + PADW))
        imTa = imTa.astype(ml_dtypes.float8_e4m3 if FP8CTX
                           else ml_dtypes.bfloat16)
        selEO = np.zeros((16, 2), np.float32)
        selEO[0::2, 0] = 1.0
        selEO[1::2, 1] = 1.0
        in_maps.append({
            "selEO": selEO,
            "imB": np.ascontiguousarray(imc),
            "imTa": imTa,
            "w1T": w1T, "vb": vbf, "Wg": Wg, "w2T": w2T,
            "linTb": linTb.astype(np.float32),
            "linsel": linsel.astype(np.float32),
            "gemb": gemb,
            "id100": np.eye(100, dtype=np.float32),
            "h0aug": h0aug, "c0": c0.reshape(100, 1).astype(np.float32),
        })
    return in_maps


def finish_loss(Sout, selout):
    """loss = sum_t ( log(sum_j exp(logit_j)) - logit_sel )."""
    S = np.asarray(Sout, np.float64).ravel()
    sel = np.asarray(selout, np.float64).ravel()
    return np.float32(np.sum(np.log(S) - sel))


# =================== self-contained runner ===================
LC = 8192
T_STEPS = 258
FP8CTX = True
_CACHE = {}


def _get_compiled():
    if "nc" not in _CACHE:
        nc = build_kernel(LC, T_STEPS, fp8ctx=FP8CTX)
        nc.compile()
        _CACHE["nc"] = nc
    return _CACHE["nc"]


def kernel(**inputs):
    """Full-input AttnLSTM decoder loss on 8 trn2 cores."""
    from concourse import bass_utils
    nc = _get_compiled()
    in_maps = prep_inputs(inputs, LC, T_STEPS)
    res = bass_utils.run_bass_kernel_spmd(nc, in_maps,
                                          core_ids=list(range(NCORES)))
    out = res.results[0]
    return np.asarray(finish_loss(out["Sout"], out["selout"]))



# revision 51
# speedup vs baseline: 1.0767x; 1.0767x over previous
"""AttnLSTMDecoder Trainium2 kernel: builder + host preprocessing.

Sharding: encoder length axis L split evenly across 8 cores; per-step
softmax normalizer + context partials all-reduced via remote SBUF DMA
broadcast (mesh all-to-all, one hop). LSTM replicated on every core.
"""
import sys
sys.path.insert(0, '/opt/trn_rl_repo')
import numpy as np
import ml_dtypes
from contextlib import ExitStack
from concourse import bass, bacc, tile
mybir = bass.mybir

F32 = mybir.dt.float32
BF16 = mybir.dt.bfloat16
F8 = mybir.dt.float8e4
DR = mybir.MatmulPerfMode.DoubleRow
Tanh = mybir.ActivationFunctionType.Tanh
Exp = mybir.ActivationFunctionType.Exp
ADD = mybir.AluOpType.add
MULT = mybir.AluOpType.mult

STATE = 100
ATT = 100
EMB = 100
VOCAB = 128
NCORES = 8


def build_kernel(Lc, T, n_tanh_chunks=4, ctx_groups=4, repeats=1, ablate_exchange=False, skip=(), wbufs=2, exchange="collective", fp8ctx=True):
    """Build the per-core SPMD kernel. Lc = L/8 (multiple of 512).

    exchange: "rdma" (SBUF-to-SBUF remote DMA all-gather), "collective"
    (ncfw AllGather via HBM bounce), or "none" (ablation; wrong result).
    """
    NCH = Lc // 128          # l-chunks of 128
    assert Lc % 512 == 0
    assert NCH % n_tanh_chunks == 0
    assert NCH % ctx_groups == 0 or True
    if ablate_exchange:
        exchange = "none"
    if fp8ctx:
        assert exchange == "collective"
    nc = bacc.Bacc("TRN2", target_bir_lowering=False, debug=False,
                   num_devices=NCORES)

    # ---------------- DRAM parameters (per-core) ----------------
    d_imB = nc.declare_dram_parameter("imB", [200, Lc], F32, isOutput=False)
    d_imTa = nc.declare_dram_parameter("imTa",
                                       [128, NCH * (224 if fp8ctx else 208)],
                                       F8 if fp8ctx else BF16, isOutput=False)
    d_w1T = nc.declare_dram_parameter("w1T", [200, ATT], F32, isOutput=False)
    d_vb = nc.declare_dram_parameter("vb", [ATT, 1], BF16, isOutput=False)
    d_Wg = nc.declare_dram_parameter("Wg", [401, 400], F32, isOutput=False)
    d_w2T = nc.declare_dram_parameter("w2T", [200, ATT], F32, isOutput=False)
    d_linTb = nc.declare_dram_parameter("linTb", [101, VOCAB], F32, isOutput=False)
    d_linsel = nc.declare_dram_parameter("linsel", [101, T], F32, isOutput=False)
    d_gemb = nc.declare_dram_parameter("gemb", [100, 4 * T], F32, isOutput=False)
    d_h0 = nc.declare_dram_parameter("h0aug", [101, 1], F32, isOutput=False)
    d_c0 = nc.declare_dram_parameter("c0", [STATE, 1], F32, isOutput=False)
    d_id = nc.declare_dram_parameter("id100", [100, 100], F32, isOutput=False)
    d_S = nc.declare_dram_parameter("Sout", [1, T], F32, isOutput=True)
    d_sel = nc.declare_dram_parameter("selout", [1, T], F32, isOutput=True)
    d_selEO = nc.declare_dram_parameter("selEO", [16, 2], F32, isOutput=False)
    cc_shape = ([1, 208], [8, 208]) if fp8ctx else ([2, 416], [16, 416])
    ccin = [nc.dram_tensor(f"ccin{i}", cc_shape[0], F32) for i in range(2)]
    ccout = [nc.dram_tensor(f"ccout{i}", cc_shape[1], F32, addr_space="Shared")
             for i in range(2)]
    if exchange == "rdma":
        # per-sender-slot remote sems, double-buffered by step parity (the
        # k+2-vs-k chain is provably ordered; k+1-vs-k is not) + local
        # send-completion sem; cleared at entry before the barrier
        rsems = [[nc.alloc_semaphore(f"rsem{p}_{d}") for d in range(1, NCORES)]
                 for p in range(2)]
        lsem = nc.alloc_semaphore("lsem")
        ccbar_in = nc.dram_tensor("ccbar_in", [1, 1], mybir.dt.uint8)
        ccbar_out = nc.dram_tensor("ccbar_out", [NCORES, 1], mybir.dt.uint8,
                                   addr_space="Shared")

    ext_waits = []   # (BassInstruction, sem, value) attached post-scheduling
    with tile.TileContext(nc) as tc, ExitStack() as ctxs:
        # pools
        P = ctxs.enter_context(tc.tile_pool(name="static", bufs=1))
        W = ctxs.enter_context(tc.tile_pool(name="work", bufs=wbufs))
        PS = ctxs.enter_context(tc.tile_pool(name="psum", bufs=1,
                                             space="PSUM"))

        # ---------------- static SBUF tiles ----------------
        imB1 = P.tile([100, Lc], F32, tag="imB1")
        imB2 = P.tile([100, Lc], F32, tag="imB2")
        imTa = P.tile([128, NCH * (224 if fp8ctx else 208)],
                      F8 if fp8ctx else BF16, tag="imTa")
        sel8 = P.tile([8, 1], F32, tag="sel8")
        w1Ta = P.tile([100, ATT], F32, tag="w1Ta")
        w1Tb = P.tile([100, ATT], F32, tag="w1Tb")
        vb = P.tile([ATT, 1], BF16, tag="vb")
        Wg_ctx1 = P.tile([128, 400], F32, tag="Wgc1")
        Wg_ctx2 = P.tile([72, 400], F32, tag="Wgc2")
        Wg_h = P.tile([100, 400], F32, tag="Wgh")
        gemb = P.tile([100, 4 * T], F32, tag="gemb")
        id100 = P.tile([100, 100], F32, tag="id100")
        w2Th = P.tile([100, ATT], F32, tag="w2Th")
        w2Tc = P.tile([100, ATT], F32, tag="w2Tc")
        linTb = P.tile([101, VOCAB], F32, tag="linTb")
        linsel = P.tile([101, T], F32, tag="linsel")
        h_aug = P.tile([101, 1], F32, tag="haug")
        c_sb = P.tile([STATE, 1], F32, tag="c")
        w1tb = P.tile([ATT, Lc], BF16, tag="w1tb")
        tanh_sb = P.tile([ATT, Lc], BF16, tag="tanhsb")
        Sbuf = P.tile([1, T], F32, tag="Sbuf")
        selbuf = P.tile([1, T], F32, tag="selbuf")
        ones1 = P.tile([1, 1], F32, tag="ones1")
        ones128 = P.tile([1, 128], F32, tag="ones128")
        selEO = P.tile([16, 2], F32, tag="selEO")
        if exchange == "rdma":
            # ping-pong send payload + gathered slots: cols 0=ctx_a(128),
            # 1=den, 2=ctx_b(rows 0:72); 8 f32 per slot for 32B transfers
            flat = [P.tile([128, 8], F32, tag=f"flat{i}", name=f"flat{i}")
                    for i in range(2)]
            gf = [P.tile([128, 8 * NCORES], F32, tag=f"gf{i}", name=f"gf{i}")
                  for i in range(2)]

        # ---------------- init ----------------
        nc.sync.dma_start(imB1[:], d_imB[0:100, :])
        nc.sync.dma_start(imB2[:], d_imB[100:200, :])
        nc.sync.dma_start(imTa[:], d_imTa[:])
        nc.sync.dma_start(w1Ta[:], d_w1T[0:100, :])
        nc.sync.dma_start(w1Tb[:], d_w1T[100:200, :])
        nc.sync.dma_start(vb[:], d_vb[:])
        nc.sync.dma_start(Wg_ctx1[:], d_Wg[0:128, :])
        nc.sync.dma_start(Wg_ctx2[:], d_Wg[128:200, :])
        nc.sync.dma_start(Wg_h[:], d_Wg[301:401, :])
        nc.sync.dma_start(gemb[:], d_gemb[:])
        nc.sync.dma_start(id100[:], d_id[:])
        nc.sync.dma_start(w2Th[:], d_w2T[0:100, :])
        nc.sync.dma_start(w2Tc[:], d_w2T[100:200, :])
        nc.sync.dma_start(linTb[:], d_linTb[:])
        nc.sync.dma_start(linsel[:], d_linsel[:])
        nc.sync.dma_start(h_aug[:], d_h0[:])
        nc.sync.dma_start(c_sb[:], d_c0[:])
        nc.gpsimd.memset(ones1[:], 1.0)
        nc.gpsimd.memset(ones128[:], 1.0)
        nc.gpsimd.memset(sel8[:], 1.0)
        nc.sync.dma_start(selEO[:], d_selEO[:])
        if exchange == "rdma":
            # no gf memset: every slot byte is remotely/self written each step
            # before the tree reads it, and a local init write would look like
            # a cross-core race to the detector (collectives carry no
            # happens-before watermarks)
            for i in range(2):
                nc.gpsimd.memset(flat[i][:], 0.0)
            # clear exchange sems, THEN barrier: a peer's first send can only
            # follow its own barrier-completion, which needs our arrival,
            # which follows our clears — so no increment is ever lost
            for s in rsems[0] + rsems[1] + [lsem]:
                nc.gpsimd.sem_clear(s)
            nc.gpsimd.collective_compute(
                "AllGather", mybir.AluOpType.bypass,
                replica_groups=[list(range(NCORES))],
                ins=[ccbar_in.ap().opt()], outs=[ccbar_out.ap().opt()])

            def issue_preps(parity):
                # stage the 7 send-descriptor frames for the NEXT trigger;
                # desc-gen runs on the Pool Q7 during the tanh phase, and the
                # payload read happens at trigger time
                for d in range(1, NCORES):
                    rdest = [None] * NCORES
                    rdest[d] = (0, d)
                    nc.gpsimd.remote_dma_broadcast(
                        gf[parity][:, 8 * d:8 * d + 8], flat[parity][:, 0:8],
                        rsems[parity][d - 1], lsem, rdests=rdest)
            issue_preps(0)

        # w1t = w1 @ input_mat   -> [ATT, Lc] bf16
        for j in range(Lc // 512):
            w1p = PS.tile([ATT, 512], F32, tag="w1p")
            sl = slice(512 * j, 512 * (j + 1))
            nc.tensor.matmul(w1p[:], w1Ta[:], imB1[:, sl], start=True, stop=False)
            nc.tensor.matmul(w1p[:], w1Tb[:], imB2[:, sl], start=False, stop=True)
            nc.scalar.copy(w1tb[:, sl], w1p[:])

        CH = NCH // n_tanh_chunks  # l-chunks per tanh chunk
        # tapered chunk bounds: a small final chunk shortens the PE trail
        # (scores+ctx of the last chunk run after the last tanh finishes)
        if n_tanh_chunks == 4 and NCH % 16 == 0 and NCH >= 32:
            tail = NCH // 16
            big = (NCH - tail + 2) // 3
            bounds = [0, big, 2 * big, NCH - tail, NCH]
        else:
            bounds = [i * CH for i in range(n_tanh_chunks + 1)]

        # ---------------- decode steps ----------------
        def emit_logits(tt):
            lg_full = PS.tile([1, 512], F32, tag="lg")
            lg_ps = lg_full[:, 0:129]
            nc.tensor.matmul(lg_ps[0:1, 0:128], h_aug[:, 0:1], linTb[:],
                             start=True, stop=True)
            nc.tensor.matmul(lg_ps[0:1, 128:129], h_aug[:, 0:1],
                             linsel[:, tt:tt + 1], start=True, stop=True)
            exps = W.tile([1, VOCAB], F32, tag="exps")
            nc.scalar.activation(exps[:], lg_ps[0:1, 0:128], Exp,
                                 accum_out=Sbuf[0:1, tt:tt + 1])
            nc.vector.tensor_copy(selbuf[0:1, tt:tt + 1],
                                  lg_ps[0:1, 128:129])

        steps = [tt for _ in range(repeats) for tt in range(T)]
        for k, t in enumerate(steps):
            # w2dt = w2 @ [h; c]  -> bias for tanh
            w2p_full = PS.tile([ATT, 512], F32, tag="w2p")
            w2p = w2p_full[:, 0:1]
            nc.tensor.matmul(w2p[:], w2Th[:], h_aug[0:100, 0:1], start=True, stop=False)
            nc.tensor.matmul(w2p[:], w2Tc[:], c_sb[:], start=False, stop=True)
            bias_sb = W.tile([ATT, 1], F32, tag="bias")
            nc.scalar.copy(bias_sb[:], w2p[:])

            # gates from h + host-precomputed emb/bias part (via identity
            # matmul) — known at step start; issue early so the PE covers
            # them while ACT runs the first tanh chunk
            gates_full = PS.tile([100, 512], F32, tag="gates")
            gates_ps = gates_full[:, 0:4]
            nc.tensor.matmul(gates_ps[:], id100[:],
                             gemb[:, 4 * t:4 * t + 4], start=True, stop=False,
                             skip_group_check=True)
            for g in range(4 if "gates" not in skip else 0):
                gs = slice(100 * g, 100 * (g + 1))
                nc.tensor.matmul(gates_ps[:, g:g + 1], Wg_h[:, gs],
                                 h_aug[0:100, 0:1], start=False, stop=False,
                                 skip_group_check=True)

            if exchange == "rdma":
                gfb, flb = gf[k % 2], flat[k % 2]

            scores_full = PS.tile([128, 512], F32, tag="scores")
            scores_ps = scores_full[:, 0:NCH]
            if fp8ctx:
                # fp8 att at 16B column stride (DoubleRow weight constraint)
                att_sb = W.tile([128, NCH * 16], F8, tag="att")
                att3 = att_sb[:].rearrange("p (n s) -> p n s", s=16)
                imTa3 = imTa[:].rearrange("p (k n) -> p k n", n=224)
            else:
                att_sb = W.tile([128, NCH], BF16, tag="att")
            ctx_full = PS.tile([2, 512], F32, tag="ctx")
            ctx_ps = ctx_full[0:1, 0:208] if fp8ctx else ctx_full[:, 0:416]

            # software-pipelined emission: l-chunks in groups of GSZ; ctx of
            # group g is emitted after scores of group g+1 so the PE's
            # weight-load path (score LDWs) overlaps its streaming path
            # (ctx matmuls) via the LDW pull-ahead window, and exp runs at
            # group granularity on ACT between tanh chunks.
            GSZ = 8
            NG = NCH // GSZ              # 8 groups
            TCH = 1                      # groups per tanh chunk

            def emit_tanh(ti):
                lo, hi = ti * TCH * GSZ * 128, (ti + 1) * TCH * GSZ * 128
                nc.scalar.activation(tanh_sb[:, lo:hi], w1tb[:, lo:hi],
                                     Tanh, bias=bias_sb[:, 0:1])

            def emit_scores(g):
                for c in range(g * GSZ, (g + 1) * GSZ):
                    nc.tensor.matmul(scores_ps[:, c:c + 1],
                                     tanh_sb[:, c * 128:(c + 1) * 128],
                                     vb[:], start=True, stop=True)

            def emit_exp(g):
                lo, hi = g * GSZ, (g + 1) * GSZ
                if fp8ctx:
                    nc.scalar.activation(att3[:, lo:hi, 0:1],
                                         scores_ps[:, lo:hi], Exp)
                else:
                    nc.scalar.activation(att_sb[:, lo:hi],
                                         scores_ps[:, lo:hi], Exp)

            def emit_ctx(g):
                for c in range(g * GSZ, (g + 1) * GSZ, 2):
                    if fp8ctx:
                        mv = imTa3[:, c:c + 2, 0:208]
                        wv = att_sb[:, 16 * c:16 * c + 32].rearrange(
                            "p (k s) -> p k s", k=2)[:, :, 0:1]
                        nc.tensor.matmul(ctx_ps[:],
                                         wv,
                                         mv,
                                         start=(c == 0), stop=(c >= NCH - 2),
                                         perf_mode=DR, skip_group_check=True)
                    else:
                        nc.tensor.matmul(ctx_ps[:],
                                         att_sb[:, c:c + 2],
                                         imTa[:, c * 208:(c + 2) * 208],
                                         start=(c == 0), stop=(c >= NCH - 2),
                                         skip_group_check=True)

            emit_tanh(0)
            emit_tanh(1)
            emit_scores(0)
            emit_exp(0)
            for g in range(1, NG):
                if g % TCH == 0 and g // TCH + 1 < NG // TCH:
                    emit_tanh(g // TCH + 1)
                emit_scores(g)
                emit_exp(g)
                emit_ctx(g - 1)
            emit_ctx(NG - 1)

            # partial (den|ctx) rows leave PSUM uncombined
            num_sb = W.tile([1, 208] if fp8ctx else [2, 416], F32, tag="num")
            if "combine" not in skip:
                nc.vector.tensor_copy(num_sb[:], ctx_ps[:])

            rd = W.tile([128, 1], F32, tag="rd")
            ctx_sb = W.tile([128, 2], F32, tag="ctxs")
            Copy = mybir.ActivationFunctionType.Copy

            if exchange == "rdma":
                # local combine: transpose own partials to partition columns
                # cols 0=ctx_a(128), 1=den bcast, 2=ctx_b(0:72); the 0/1
                # selector columns pick the valid half-row of each segment
                sE, sO = selEO[0:2, 0:1], selEO[0:2, 1:2]
                cu_full = PS.tile([128, 512], F32, tag="cu")
                nc.tensor.matmul(cu_full[:, 0:1], num_sb[:, 1:129], sE,
                                 start=True, stop=False)
                nc.tensor.matmul(cu_full[:, 0:1], num_sb[:, 209:337], sO,
                                 start=False, stop=True)
                nc.tensor.matmul(cu_full[:, 1:2],
                                 num_sb[:, 0:1].to_broadcast((2, 128)), sE,
                                 start=True, stop=False)
                nc.tensor.matmul(cu_full[:, 1:2],
                                 num_sb[:, 208:209].to_broadcast((2, 128)), sO,
                                 start=False, stop=True)
                nc.tensor.matmul(cu_full[0:72, 2:3], num_sb[:, 129:201], sE,
                                 start=True, stop=False)
                nc.tensor.matmul(cu_full[0:72, 2:3], num_sb[:, 337:409], sO,
                                 start=False, stop=True)
                # stage the send payload (don't overwrite until the sends that
                # last used this buffer have drained; the wait is attached
                # post-scheduling so the single-core tile pass can't deadlock)
                inst = nc.vector.tensor_copy(flb[:, 0:2], cu_full[:, 0:2])
                if k >= 2:
                    # all sends through step k-1 drained (per-step completions
                    # interleave across lanes, so only full-prefix counts are
                    # provable thresholds); covers this buffer's k-2 sends
                    ext_waits.append((inst, lsem, 112 * k))
                nc.vector.tensor_copy(flb[0:72, 2:3], cu_full[0:72, 2:3])
                nc.vector.tensor_copy(gfb[:, 0:8], flb[:, 0:8])  # self slot
                # fire the frames staged last step; the declared write of the
                # payload region (WAW vs the copies) makes tile order the
                # trigger after them and prove it with an engine sem
                nc.gpsimd.trigger_dma(count=None,
                                      signals_writable=[flb[:, 0:3]])
                if k + 1 < len(steps):
                    issue_preps((k + 1) % 2)
                if "post" in skip:
                    continue
                # wait for all 7 peers' step-k payloads, then tree-reduce the
                # 8 slots in place; col 0=ctx_a, 1=den, 2=ctx_b
                inst = nc.vector.tensor_tensor(gfb[:, 0:32], gfb[:, 0:32],
                                               gfb[:, 32:64], op=ADD)
                for d in range(1, NCORES):
                    ext_waits.append((inst, rsems[k % 2][d - 1],
                                      2 * (k // 2 + 1)))
                nc.vector.tensor_tensor(gfb[:, 0:16], gfb[:, 0:16],
                                        gfb[:, 16:32], op=ADD)
                nc.vector.tensor_tensor(gfb[:, 0:8], gfb[:, 0:8],
                                        gfb[:, 8:16], op=ADD)
                nc.vector.reciprocal(rd[:], gfb[:, 1:2])
                nc.scalar.activation(ctx_sb[:, 0:1], gfb[:, 0:1], Copy,
                                     scale=rd[:, 0:1])
                nc.scalar.activation(ctx_sb[0:72, 1:2], gfb[0:72, 2:3], Copy,
                                     scale=rd[0:72, 0:1])
            else:
                # ---- exchange: AllGather the partial rows ----
                gather = W.tile([8, 208] if fp8ctx else [16, 416], F32,
                                tag="gather")
                if exchange == "none":
                    nc.vector.tensor_copy(gather[0:2, :], num_sb[:])
                else:
                    cin, cout = ccin[t % 2], ccout[t % 2]
                    nc.sync.dma_start(cin[:], num_sb[:])
                    nc.gpsimd.collective_compute(
                        "AllGather", mybir.AluOpType.bypass,
                        replica_groups=[list(range(NCORES))],
                        ins=[cin.ap().opt()], outs=[cout.ap().opt()])
                    # previous step's logits fill the collective dead window
                    if k > 0:
                        emit_logits(steps[k - 1])
                    nc.sync.dma_start(gather[:], cout[:])

                if "post" in skip:
                    continue
                if fp8ctx:
                    # reduce the 8 gathered [1,208] rows AND transpose to
                    # partition columns in 3 matmuls with a ones selector
                    cu_full = PS.tile([128, 512], F32, tag="cu")
                    g_ = gather[0:8, :]
                    nc.tensor.matmul(cu_full[:, 0:1], g_[:, 1:129], sel8[:],
                                     start=True, stop=True)
                    nc.tensor.matmul(cu_full[0:72, 1:2], g_[:, 129:201],
                                     sel8[:], start=True, stop=True)
                    nc.tensor.matmul(cu_full[:, 2:3],
                                     g_[:, 0:1].to_broadcast((8, 128)),
                                     sel8[:], start=True, stop=True)
                else:
                    # reduce over ranks AND transpose to columns: even rows
                    # carry cols 0:208, odd rows cols 208:416; 0/1 masks
                    # select them into the same PSUM columns
                    KR = 2 if exchange == "none" else 16
                    sE, sO = selEO[0:KR, 0:1], selEO[0:KR, 1:2]
                    cu_full = PS.tile([128, 512], F32, tag="cu")
                    g_ = gather[0:KR, :]
                    nc.tensor.matmul(cu_full[:, 0:1], g_[:, 1:129], sE,
                                     start=True, stop=False)
                    nc.tensor.matmul(cu_full[:, 0:1], g_[:, 209:337], sO,
                                     start=False, stop=True)
                    nc.tensor.matmul(cu_full[0:72, 1:2], g_[:, 129:201], sE,
                                     start=True, stop=False)
                    nc.tensor.matmul(cu_full[0:72, 1:2], g_[:, 337:409], sO,
                                     start=False, stop=True)
                    # den reduced AND broadcast to all 128 partitions in one
                    # matmul (stationary free-dim stride 0 replicates it)
                    nc.tensor.matmul(cu_full[:, 2:3],
                                     g_[:, 0:1].to_broadcast((KR, 128)), sE,
                                     start=True, stop=False)
                    nc.tensor.matmul(cu_full[:, 2:3],
                                     g_[:, 208:209].to_broadcast((KR, 128)),
                                     sO, start=False, stop=True)
                nc.vector.reciprocal(rd[:], cu_full[:, 2:3])
                nc.scalar.activation(ctx_sb[:, 0:1], cu_full[:, 0:1], Copy,
                                     scale=rd[:, 0:1])
                nc.scalar.activation(ctx_sb[0:72, 1:2], cu_full[0:72, 1:2], Copy,
                                     scale=rd[0:72, 0:1])

            # close the gates accumulation with the ctx contributions
            for g in range(4 if "gates" not in skip else 0):
                gs = slice(100 * g, 100 * (g + 1))
                nc.tensor.matmul(gates_ps[:, g:g + 1], Wg_ctx1[:, gs],
                                 ctx_sb[:, 0:1], start=False, stop=False,
                                 skip_group_check=True)
                nc.tensor.matmul(gates_ps[:, g:g + 1], Wg_ctx2[:, gs],
                                 ctx_sb[0:72, 1:2], start=False, stop=True,
                                 skip_group_check=True)

            # LSTM elementwise
            t_all = W.tile([100, 4], F32, tag="tall")
            if "lstm" in skip:
                continue
            nc.scalar.activation(t_all[:], gates_ps[:], Tanh)
            sig = W.tile([100, 3], F32, tag="sig")
            nc.vector.tensor_scalar(sig[:], t_all[:, 0:3], 1.0, 0.5, ADD, MULT)
            tmp1 = W.tile([100, 1], F32, tag="tmp1")
            tmp2 = W.tile([100, 1], F32, tag="tmp2")
            nc.vector.tensor_tensor(tmp1[:], sig[:, 1:2], c_sb[:], op=MULT)
            nc.vector.tensor_tensor(tmp2[:], sig[:, 0:1], t_all[:, 3:4],
                                    op=MULT)
            nc.vector.tensor_tensor(c_sb[:], tmp1[:], tmp2[:], op=ADD)
            tanh_c = W.tile([100, 1], F32, tag="tanhc")
            nc.scalar.activation(tanh_c[:], c_sb[:], Tanh)
            nc.vector.tensor_tensor(h_aug[0:100, 0:1], sig[:, 2:3], tanh_c[:],
                                    op=MULT)
            if exchange != "collective" and "logits" not in skip:
                emit_logits(t)

        if exchange == "collective":
            emit_logits(steps[-1])
        nc.sync.dma_start(d_S[:], Sbuf[:])
        nc.sync.dma_start(d_sel[:], selbuf[:])

    # cross-core sem waits are invisible to the single-core tile scheduler
    # (it would deadlock); attach them to the scheduled instructions now —
    # compile()'s generate_event_semaphores legalizes multi-wait instructions
    for inst, sem, val in ext_waits:
        inst.wait_op(sem, val, "sem-ge", check=False)

    return nc


# =================== host preprocessing ===================

def _lstm_step_np(x, h, c, W_ih, W_hh, b_ih, b_hh):
    gates = W_ih @ x + b_ih + W_hh @ h + b_hh
    i, f, g, o = np.split(gates, 4)
    sig = lambda v: 1.0 / (1.0 + np.exp(-v))
    c = sig(f) * c + sig(i) * np.tanh(g)
    h = sig(o) * np.tanh(c)
    return h, c


def prep_inputs(inputs, Lc, T):
    """Produce the 8 per-core in_maps from the full problem inputs."""
    im = np.asarray(inputs["input_mat"], np.float32)        # [200, L]
    output_ids = np.asarray(inputs["output_ids"]).astype(np.int64)
    W_ih = np.asarray(inputs["W_ih"], np.float32)
    W_hh = np.asarray(inputs["W_hh"], np.float32)
    b_ih = np.asarray(inputs["b_ih"], np.float32)
    b_hh = np.asarray(inputs["b_hh"], np.float32)
    w1 = np.asarray(inputs["w1"], np.float32)
    w2 = np.asarray(inputs["w2"], np.float32)
    v_w = np.asarray(inputs["v_w"], np.float32)
    lin_w = np.asarray(inputs["lin_w"], np.float32)
    lin_b = np.asarray(inputs["lin_b"], np.float32)
    emb = np.asarray(inputs["emb"], np.float32)
    eos = int(np.asarray(inputs["eos_id"]))

    L = im.shape[1]
    assert Lc * NCORES == L and len(output_ids) == T
    NCH = Lc // 128

    # priming LSTM step on host (exact fp32 math, tiny)
    x0 = np.concatenate([np.zeros(200, np.float32), emb[eos]])
    h0, c0 = _lstm_step_np(x0, np.zeros(100, np.float32),
                           np.zeros(100, np.float32), W_ih, W_hh, b_ih, b_hh)
    h0aug = np.concatenate([h0, [1.0]]).astype(np.float32).reshape(101, 1)

    # gates weights: reorder [i,f,g,o] -> [i,f,o,g], scale i,f,o by 0.5,
    # columns [ctx(200); emb(100); bias(1); h(100)]; transposed for lhsT.
    Wcomb = np.concatenate([W_ih, W_hh], axis=1)            # [400, 400]
    bias = (b_ih + b_hh).astype(np.float32)                 # [400]
    order = np.concatenate([np.arange(100), np.arange(100, 200),
                            np.arange(300, 400), np.arange(200, 300)])
    Wr = Wcomb[order]                                       # rows i,f,o,g
    br = bias[order].copy()
    scale = np.ones((400, 1), np.float32); scale[0:300] = 0.5
    Wr = Wr * scale; br = br * scale[:, 0]
    Wg = np.zeros((401, 400), np.float32)
    # Wcomb columns = [x(300) ; h(100)] where x = [ctx(200); emb(100)]
    Wg[0:200] = Wr[:, 0:200].T       # ctx
    Wg[200:300] = Wr[:, 200:300].T   # emb
    Wg[300] = br                     # bias row
    Wg[301:401] = Wr[:, 300:400].T   # h
    # emb sequence (last_emb per step) with bias-1 row
    emb_seq = np.empty((T, EMB), np.float32)
    emb_seq[0] = emb[eos]
    emb_seq[1:] = emb[output_ids[:T - 1]]
    embaug = np.concatenate([emb_seq.T, np.ones((1, T), np.float32)], axis=0)
    # host-precomputed per-step emb+bias gate contributions:
    # gemb[p, 4t+g] = (Wg[200:301].T @ embaug)[100g+p, t]
    GE = Wg[200:301].T @ embaug                              # [400, T]
    gemb = GE.reshape(4, 100, T).transpose(1, 2, 0).reshape(100, 4 * T)
    gemb = np.ascontiguousarray(gemb, np.float32)
    # logits weights
    linTb = np.concatenate([lin_w.T, lin_b.reshape(1, -1)], axis=0)  # [101,128]
    linsel = np.concatenate([lin_w[output_ids[:T]].T,
                             lin_b[output_ids[:T]].reshape(1, -1)], axis=0)

    w1T = w1.T.copy()                 # [200, 100]
    w2T = w2.T.copy()                 # [200, 100]
    vbf = v_w.reshape(ATT, 1).astype(ml_dtypes.bfloat16)

    in_maps = []
    for cidx in range(NCORES):
        sl = slice(cidx * Lc, (cidx + 1) * Lc)
        imc = im[:, sl]                                    # [200, Lc]
        # imTa[q, c*201 + s] = im[s, c*128+q]; col 200 = 1.0
        blocks = imc.T.reshape(NCH, 128, 200)              # [c, q, s]
        PADW = 23 if FP8CTX else 7
        A = np.concatenate(
            [np.ones((NCH, 128, 1), np.float32), blocks,
             np.zeros((NCH, 128, PADW), np.float32)], axis=2)
        imTa = A.transpose(1, 0, 2).reshape(128, NCH * (201 + PADW))
        imTa = imTa.astype(ml_dtypes.float8_e4m3 if FP8CTX
                           else ml_dtypes.bfloat16)
        selEO = np.zeros((16, 2), np.float32)
        selEO[0::2, 0] = 1.0
        selEO[1::2, 1] = 1.0
        in_maps.append({
            "selEO": selEO,
            "imB": np.ascontiguousarray(imc),
            "imTa": imTa,
            "w1T": w1T, "vb": vbf, "Wg": Wg, "w2T": w2T,
            "linTb": linTb.astype(np.float32),
            "linsel": linsel.astype(np.float32),
            "gemb": gemb,
            "id100": np.eye(100, dtype=np.float32),
            "h0aug": h0aug, "c0": c0.reshape(100, 1).astype(np.float32),
        })
    return in_maps


def finish_loss(Sout, selout):
    """loss = sum_t ( log(sum_j exp(logit_j)) - logit_sel )."""
    S = np.asarray(Sout, np.float64).ravel()
    sel = np.asarray(selout, np.float64).ravel()
    return np.float32(np.sum(np.log(S) - sel))


# =================== self-contained runner ===================
LC = 8192
T_STEPS = 258
FP8CTX = True
_CACHE = {}


def _get_compiled():
    if "nc" not in _CACHE:
        nc = build_kernel(LC, T_STEPS, fp8ctx=FP8CTX)
        nc.compile()
        _CACHE["nc"] = nc
    return _CACHE["nc"]


def kernel(**inputs):
    """Full-input AttnLSTM decoder loss on 8 trn2 cores."""
    from concourse import bass_utils
    nc = _get_compiled()
    in_maps = prep_inputs(inputs, LC, T_STEPS)
    res = bass_utils.run_bass_kernel_spmd(nc, in_maps,
                                          core_ids=list(range(NCORES)))
    out = res.results[0]
    return np.asarray(finish_loss(out["Sout"], out["selout"]))



# revision 56
# speedup vs baseline: 1.0815x; 1.0045x over previous
"""AttnLSTMDecoder Trainium2 kernel: builder + host preprocessing.

Sharding: encoder length axis L split evenly across 8 cores; per-step
softmax normalizer + context partials combined via an ncfw AllGather
(HBM bounce) + on-PE reduce. LSTM replicated on every core.

Per-step structure: tanh(w1dt + w2dt) in chunks on ACT, softmax scores
via per-chunk stationary matmuls, attention context via fp8 DoubleRow
matmuls (2 l-chunks per MM), emission software-pipelined so the PE
weight-load path (score LDWs) overlaps its streaming path (ctx MMs).
Embedding-side gate contributions are host-precomputed; logits run in
the collective dead window.
"""
import sys
sys.path.insert(0, '/opt/trn_rl_repo')
import numpy as np
import ml_dtypes
from contextlib import ExitStack
from concourse import bass, bacc, tile
mybir = bass.mybir

F32 = mybir.dt.float32
BF16 = mybir.dt.bfloat16
F8 = mybir.dt.float8e4
DR = mybir.MatmulPerfMode.DoubleRow
Tanh = mybir.ActivationFunctionType.Tanh
Exp = mybir.ActivationFunctionType.Exp
ADD = mybir.AluOpType.add
MULT = mybir.AluOpType.mult

STATE = 100
ATT = 100
EMB = 100
VOCAB = 128
NCORES = 8


def build_kernel(Lc, T, n_tanh_chunks=4, ctx_groups=4, repeats=1, ablate_exchange=False, skip=(), wbufs=2, exchange="collective", fp8ctx=True):
    """Build the per-core SPMD kernel. Lc = L/8 (multiple of 512).

    exchange: "rdma" (SBUF-to-SBUF remote DMA all-gather), "collective"
    (ncfw AllGather via HBM bounce), or "none" (ablation; wrong result).
    """
    NCH = Lc // 128          # l-chunks of 128
    assert Lc % 512 == 0
    assert NCH % n_tanh_chunks == 0
    assert NCH % ctx_groups == 0 or True
    if ablate_exchange:
        exchange = "none"
    if fp8ctx:
        assert exchange == "collective"
    nc = bacc.Bacc("TRN2", target_bir_lowering=False, debug=False,
                   num_devices=NCORES)

    # ---------------- DRAM parameters (per-core) ----------------
    d_imB = nc.declare_dram_parameter("imB", [200, Lc], F32, isOutput=False)
    d_imTa = nc.declare_dram_parameter("imTa",
                                       [128, NCH * (224 if fp8ctx else 208)],
                                       F8 if fp8ctx else BF16, isOutput=False)
    d_w1T = nc.declare_dram_parameter("w1T", [200, ATT], F32, isOutput=False)
    d_vb = nc.declare_dram_parameter("vb", [ATT, 1], BF16, isOutput=False)
    d_Wg = nc.declare_dram_parameter("Wg", [401, 400], F32, isOutput=False)
    d_w2T = nc.declare_dram_parameter("w2T", [200, ATT], F32, isOutput=False)
    d_linTb = nc.declare_dram_parameter("linTb", [101, VOCAB], F32, isOutput=False)
    d_linsel = nc.declare_dram_parameter("linsel", [101, T], F32, isOutput=False)
    d_gemb = nc.declare_dram_parameter("gemb", [100, 4 * T], F32, isOutput=False)
    d_h0 = nc.declare_dram_parameter("h0aug", [101, 1], F32, isOutput=False)
    d_c0 = nc.declare_dram_parameter("c0", [STATE, 1], F32, isOutput=False)
    d_id = nc.declare_dram_parameter("id100", [100, 100], F32, isOutput=False)
    d_S = nc.declare_dram_parameter("Sout", [1, T], F32, isOutput=True)
    d_sel = nc.declare_dram_parameter("selout", [1, T], F32, isOutput=True)
    d_selEO = nc.declare_dram_parameter("selEO", [16, 2], F32, isOutput=False)
    cc_shape = ([1, 208], [8, 208]) if fp8ctx else ([2, 416], [16, 416])
    ccin = [nc.dram_tensor(f"ccin{i}", cc_shape[0], F32) for i in range(2)]
    ccout = [nc.dram_tensor(f"ccout{i}", cc_shape[1], F32, addr_space="Shared")
             for i in range(2)]
    if exchange == "rdma":
        # per-sender-slot remote sems, double-buffered by step parity (the
        # k+2-vs-k chain is provably ordered; k+1-vs-k is not) + local
        # send-completion sem; cleared at entry before the barrier
        rsems = [[nc.alloc_semaphore(f"rsem{p}_{d}") for d in range(1, NCORES)]
                 for p in range(2)]
        lsem = nc.alloc_semaphore("lsem")
        ccbar_in = nc.dram_tensor("ccbar_in", [1, 1], mybir.dt.uint8)
        ccbar_out = nc.dram_tensor("ccbar_out", [NCORES, 1], mybir.dt.uint8,
                                   addr_space="Shared")

    ext_waits = []   # (BassInstruction, sem, value) attached post-scheduling
    with tile.TileContext(nc) as tc, ExitStack() as ctxs:
        # pools
        P = ctxs.enter_context(tc.tile_pool(name="static", bufs=1))
        W = ctxs.enter_context(tc.tile_pool(name="work", bufs=wbufs))
        PS = ctxs.enter_context(tc.tile_pool(name="psum", bufs=1,
                                             space="PSUM"))

        # ---------------- static SBUF tiles ----------------
        imB1 = P.tile([100, Lc], F32, tag="imB1")
        imB2 = P.tile([100, Lc], F32, tag="imB2")
        imTa = P.tile([128, NCH * (224 if fp8ctx else 208)],
                      F8 if fp8ctx else BF16, tag="imTa")
        sel8 = P.tile([8, 1], F32, tag="sel8")
        w1Ta = P.tile([100, ATT], F32, tag="w1Ta")
        w1Tb = P.tile([100, ATT], F32, tag="w1Tb")
        vb = P.tile([ATT, 1], BF16, tag="vb")
        Wg_ctx1 = P.tile([128, 400], F32, tag="Wgc1")
        Wg_ctx2 = P.tile([72, 400], F32, tag="Wgc2")
        Wg_h = P.tile([100, 400], F32, tag="Wgh")
        gemb = P.tile([100, 4 * T], F32, tag="gemb")
        id100 = P.tile([100, 100], F32, tag="id100")
        w2Th = P.tile([100, ATT], F32, tag="w2Th")
        w2Tc = P.tile([100, ATT], F32, tag="w2Tc")
        linTb = P.tile([101, VOCAB], F32, tag="linTb")
        linsel = P.tile([101, T], F32, tag="linsel")
        h_aug = P.tile([101, 1], F32, tag="haug")
        c_sb = P.tile([STATE, 1], F32, tag="c")
        w1tb = P.tile([ATT, Lc], BF16, tag="w1tb")
        tanh_sb = P.tile([ATT, Lc], BF16, tag="tanhsb")
        Sbuf = P.tile([1, T], F32, tag="Sbuf")
        selbuf = P.tile([1, T], F32, tag="selbuf")
        wkA = P.tile([128, 1024], F32, tag="wkA")
        wkB = P.tile([128, 1024], F32, tag="wkB")
        ones1 = P.tile([1, 1], F32, tag="ones1")
        ones128 = P.tile([1, 128], F32, tag="ones128")
        selEO = P.tile([16, 2], F32, tag="selEO")
        if exchange == "rdma":
            # ping-pong send payload + gathered slots: cols 0=ctx_a(128),
            # 1=den, 2=ctx_b(rows 0:72); 8 f32 per slot for 32B transfers
            flat = [P.tile([128, 8], F32, tag=f"flat{i}", name=f"flat{i}")
                    for i in range(2)]
            gf = [P.tile([128, 8 * NCORES], F32, tag=f"gf{i}", name=f"gf{i}")
                  for i in range(2)]

        # ---------------- init ----------------
        nc.sync.dma_start(imB1[:], d_imB[0:100, :])
        nc.sync.dma_start(imB2[:], d_imB[100:200, :])
        nc.sync.dma_start(imTa[:], d_imTa[:])
        nc.sync.dma_start(w1Ta[:], d_w1T[0:100, :])
        nc.sync.dma_start(w1Tb[:], d_w1T[100:200, :])
        nc.sync.dma_start(vb[:], d_vb[:])
        nc.sync.dma_start(Wg_ctx1[:], d_Wg[0:128, :])
        nc.sync.dma_start(Wg_ctx2[:], d_Wg[128:200, :])
        nc.sync.dma_start(Wg_h[:], d_Wg[301:401, :])
        nc.sync.dma_start(gemb[:], d_gemb[:])
        nc.sync.dma_start(id100[:], d_id[:])
        nc.sync.dma_start(w2Th[:], d_w2T[0:100, :])
        nc.sync.dma_start(w2Tc[:], d_w2T[100:200, :])
        nc.sync.dma_start(linTb[:], d_linTb[:])
        nc.sync.dma_start(linsel[:], d_linsel[:])
        nc.sync.dma_start(h_aug[:], d_h0[:])
        nc.sync.dma_start(c_sb[:], d_c0[:])
        nc.gpsimd.memset(ones1[:], 1.0)
        nc.gpsimd.memset(ones128[:], 1.0)
        nc.gpsimd.memset(sel8[:], 1.0)
        nc.gpsimd.memset(wkA[:], 0.5)
        nc.gpsimd.memset(wkB[:], 0.5)
        nc.sync.dma_start(selEO[:], d_selEO[:])
        if exchange == "rdma":
            # no gf memset: every slot byte is remotely/self written each step
            # before the tree reads it, and a local init write would look like
            # a cross-core race to the detector (collectives carry no
            # happens-before watermarks)
            for i in range(2):
                nc.gpsimd.memset(flat[i][:], 0.0)
            # clear exchange sems, THEN barrier: a peer's first send can only
            # follow its own barrier-completion, which needs our arrival,
            # which follows our clears — so no increment is ever lost
            for s in rsems[0] + rsems[1] + [lsem]:
                nc.gpsimd.sem_clear(s)
            nc.gpsimd.collective_compute(
                "AllGather", mybir.AluOpType.bypass,
                replica_groups=[list(range(NCORES))],
                ins=[ccbar_in.ap().opt()], outs=[ccbar_out.ap().opt()])

            def issue_preps(parity):
                # stage the 7 send-descriptor frames for the NEXT trigger;
                # desc-gen runs on the Pool Q7 during the tanh phase, and the
                # payload read happens at trigger time
                for d in range(1, NCORES):
                    rdest = [None] * NCORES
                    rdest[d] = (0, d)
                    nc.gpsimd.remote_dma_broadcast(
                        gf[parity][:, 8 * d:8 * d + 8], flat[parity][:, 0:8],
                        rsems[parity][d - 1], lsem, rdests=rdest)
            issue_preps(0)

        # w1t = w1 @ input_mat   -> [ATT, Lc] bf16
        for j in range(Lc // 512):
            w1p = PS.tile([ATT, 512], F32, tag="w1p")
            sl = slice(512 * j, 512 * (j + 1))
            nc.tensor.matmul(w1p[:], w1Ta[:], imB1[:, sl], start=True, stop=False)
            nc.tensor.matmul(w1p[:], w1Tb[:], imB2[:, sl], start=False, stop=True)
            nc.scalar.copy(w1tb[:, sl], w1p[:])

        CH = NCH // n_tanh_chunks  # l-chunks per tanh chunk
        # tapered chunk bounds: a small final chunk shortens the PE trail
        # (scores+ctx of the last chunk run after the last tanh finishes)
        if n_tanh_chunks == 4 and NCH % 16 == 0 and NCH >= 32:
            tail = NCH // 16
            big = (NCH - tail + 2) // 3
            bounds = [0, big, 2 * big, NCH - tail, NCH]
        else:
            bounds = [i * CH for i in range(n_tanh_chunks + 1)]

        # ---------------- decode steps ----------------
        def emit_logits(tt):
            lg_full = PS.tile([1, 512], F32, tag="lg")
            lg_ps = lg_full[:, 0:129]
            nc.tensor.matmul(lg_ps[0:1, 0:128], h_aug[:, 0:1], linTb[:],
                             start=True, stop=True)
            nc.tensor.matmul(lg_ps[0:1, 128:129], h_aug[:, 0:1],
                             linsel[:, tt:tt + 1], start=True, stop=True)
            exps = W.tile([1, VOCAB], F32, tag="exps")
            nc.scalar.activation(exps[:], lg_ps[0:1, 0:128], Exp,
                                 accum_out=Sbuf[0:1, tt:tt + 1])
            nc.vector.tensor_copy(selbuf[0:1, tt:tt + 1],
                                  lg_ps[0:1, 128:129])

        steps = [tt for _ in range(repeats) for tt in range(T)]
        for k, t in enumerate(steps):
            # w2dt = w2 @ [h; c]  -> bias for tanh
            w2p_full = PS.tile([ATT, 512], F32, tag="w2p")
            w2p = w2p_full[:, 0:1]
            nc.tensor.matmul(w2p[:], w2Th[:], h_aug[0:100, 0:1], start=True, stop=False)
            nc.tensor.matmul(w2p[:], w2Tc[:], c_sb[:], start=False, stop=True)
            bias_sb = W.tile([ATT, 1], F32, tag="bias")
            nc.scalar.copy(bias_sb[:], w2p[:])

            # gates from h + host-precomputed emb/bias part (via identity
            # matmul) — known at step start; issue early so the PE covers
            # them while ACT runs the first tanh chunk
            gates_full = PS.tile([100, 512], F32, tag="gates")
            gates_ps = gates_full[:, 0:4]
            nc.tensor.matmul(gates_ps[:], id100[:],
                             gemb[:, 4 * t:4 * t + 4], start=True, stop=False,
                             skip_group_check=True)
            for g in range(4 if "gates" not in skip else 0):
                gs = slice(100 * g, 100 * (g + 1))
                nc.tensor.matmul(gates_ps[:, g:g + 1], Wg_h[:, gs],
                                 h_aug[0:100, 0:1], start=False, stop=False,
                                 skip_group_check=True)

            if exchange == "rdma":
                gfb, flb = gf[k % 2], flat[k % 2]

            scores_full = PS.tile([128, 512], F32, tag="scores")
            scores_ps = scores_full[:, 0:NCH]
            if fp8ctx:
                # fp8 att at 16B column stride (DoubleRow weight constraint)
                att_sb = W.tile([128, NCH * 16], F8, tag="att")
                att3 = att_sb[:].rearrange("p (n s) -> p n s", s=16)
                imTa3 = imTa[:].rearrange("p (k n) -> p k n", n=224)
            else:
                att_sb = W.tile([128, NCH], BF16, tag="att")
            ctx_full = PS.tile([2, 512], F32, tag="ctx")
            ctx_ps = ctx_full[0:1, 0:208] if fp8ctx else ctx_full[:, 0:416]

            # software-pipelined emission: l-chunks in groups of GSZ; ctx of
            # group g is emitted after scores of group g+1 so the PE's
            # weight-load path (score LDWs) overlaps its streaming path
            # (ctx matmuls) via the LDW pull-ahead window, and exp runs at
            # group granularity on ACT between tanh chunks.
            GSZ = 8
            NG = NCH // GSZ              # 8 groups
            TCH = 1                      # groups per tanh chunk

            def emit_tanh(ti):
                lo, hi = ti * TCH * GSZ * 128, (ti + 1) * TCH * GSZ * 128
                nc.scalar.activation(tanh_sb[:, lo:hi], w1tb[:, lo:hi],
                                     Tanh, bias=bias_sb[:, 0:1])

            def emit_scores(g):
                for c in range(g * GSZ, (g + 1) * GSZ):
                    nc.tensor.matmul(scores_ps[:, c:c + 1],
                                     tanh_sb[:, c * 128:(c + 1) * 128],
                                     vb[:], start=True, stop=True)

            def emit_exp(g):
                lo, hi = g * GSZ, (g + 1) * GSZ
                if fp8ctx:
                    nc.scalar.activation(att3[:, lo:hi, 0:1],
                                         scores_ps[:, lo:hi], Exp)
                else:
                    nc.scalar.activation(att_sb[:, lo:hi],
                                         scores_ps[:, lo:hi], Exp)

            def emit_ctx(g):
                for c in range(g * GSZ, (g + 1) * GSZ, 2):
                    if fp8ctx:
                        mv = imTa3[:, c:c + 2, 0:208]
                        wv = att_sb[:, 16 * c:16 * c + 32].rearrange(
                            "p (k s) -> p k s", k=2)[:, :, 0:1]
                        nc.tensor.matmul(ctx_ps[:],
                                         wv,
                                         mv,
                                         start=(c == 0), stop=(c >= NCH - 2),
                                         perf_mode=DR, skip_group_check=True)
                    else:
                        nc.tensor.matmul(ctx_ps[:],
                                         att_sb[:, c:c + 2],
                                         imTa[:, c * 208:(c + 2) * 208],
                                         start=(c == 0), stop=(c >= NCH - 2),
                                         skip_group_check=True)

            emit_tanh(0)
            emit_tanh(1)
            emit_scores(0)
            emit_exp(0)
            for g in range(1, NG):
                if g % TCH == 0 and g // TCH + 1 < NG // TCH:
                    emit_tanh(g // TCH + 1)
                emit_scores(g)
                emit_exp(g)
                emit_ctx(g - 1)
            emit_ctx(NG - 1)

            # partial (den|ctx) rows leave PSUM uncombined
            num_sb = W.tile([1, 208] if fp8ctx else [2, 416], F32, tag="num")
            if "combine" not in skip:
                nc.vector.tensor_copy(num_sb[:], ctx_ps[:])

            rd = W.tile([128, 1], F32, tag="rd")
            ctx_sb = W.tile([128, 2], F32, tag="ctxs")
            Copy = mybir.ActivationFunctionType.Copy

            if exchange == "rdma":
                # local combine: transpose own partials to partition columns
                # cols 0=ctx_a(128), 1=den bcast, 2=ctx_b(0:72); the 0/1
                # selector columns pick the valid half-row of each segment
                sE, sO = selEO[0:2, 0:1], selEO[0:2, 1:2]
                cu_full = PS.tile([128, 512], F32, tag="cu")
                nc.tensor.matmul(cu_full[:, 0:1], num_sb[:, 1:129], sE,
                                 start=True, stop=False)
                nc.tensor.matmul(cu_full[:, 0:1], num_sb[:, 209:337], sO,
                                 start=False, stop=True)
                nc.tensor.matmul(cu_full[:, 1:2],
                                 num_sb[:, 0:1].to_broadcast((2, 128)), sE,
                                 start=True, stop=False)
                nc.tensor.matmul(cu_full[:, 1:2],
                                 num_sb[:, 208:209].to_broadcast((2, 128)), sO,
                                 start=False, stop=True)
                nc.tensor.matmul(cu_full[0:72, 2:3], num_sb[:, 129:201], sE,
                                 start=True, stop=False)
                nc.tensor.matmul(cu_full[0:72, 2:3], num_sb[:, 337:409], sO,
                                 start=False, stop=True)
                # stage the send payload (don't overwrite until the sends that
                # last used this buffer have drained; the wait is attached
                # post-scheduling so the single-core tile pass can't deadlock)
                inst = nc.vector.tensor_copy(flb[:, 0:2], cu_full[:, 0:2])
                if k >= 2:
                    # all sends through step k-1 drained (per-step completions
                    # interleave across lanes, so only full-prefix counts are
                    # provable thresholds); covers this buffer's k-2 sends
                    ext_waits.append((inst, lsem, 112 * k))
                nc.vector.tensor_copy(flb[0:72, 2:3], cu_full[0:72, 2:3])
                nc.vector.tensor_copy(gfb[:, 0:8], flb[:, 0:8])  # self slot
                # fire the frames staged last step; the declared write of the
                # payload region (WAW vs the copies) makes tile order the
                # trigger after them and prove it with an engine sem
                nc.gpsimd.trigger_dma(count=None,
                                      signals_writable=[flb[:, 0:3]])
                if k + 1 < len(steps):
                    issue_preps((k + 1) % 2)
                if "post" in skip:
                    continue
                # wait for all 7 peers' step-k payloads, then tree-reduce the
                # 8 slots in place; col 0=ctx_a, 1=den, 2=ctx_b
                inst = nc.vector.tensor_tensor(gfb[:, 0:32], gfb[:, 0:32],
                                               gfb[:, 32:64], op=ADD)
                for d in range(1, NCORES):
                    ext_waits.append((inst, rsems[k % 2][d - 1],
                                      2 * (k // 2 + 1)))
                nc.vector.tensor_tensor(gfb[:, 0:16], gfb[:, 0:16],
                                        gfb[:, 16:32], op=ADD)
                nc.vector.tensor_tensor(gfb[:, 0:8], gfb[:, 0:8],
                                        gfb[:, 8:16], op=ADD)
                nc.vector.reciprocal(rd[:], gfb[:, 1:2])
                nc.scalar.activation(ctx_sb[:, 0:1], gfb[:, 0:1], Copy,
                                     scale=rd[:, 0:1])
                nc.scalar.activation(ctx_sb[0:72, 1:2], gfb[0:72, 2:3], Copy,
                                     scale=rd[0:72, 0:1])
            else:
                # ---- exchange: AllGather the partial rows ----
                gather = W.tile([8, 208] if fp8ctx else [16, 416], F32,
                                tag="gather")
                if exchange == "none":
                    nc.vector.tensor_copy(gather[0:2, :], num_sb[:])
                else:
                    cin, cout = ccin[t % 2], ccout[t % 2]
                    nc.sync.dma_start(cin[:], num_sb[:])
                    nc.gpsimd.collective_compute(
                        "AllGather", mybir.AluOpType.bypass,
                        replica_groups=[list(range(NCORES))],
                        ins=[cin.ap().opt()], outs=[cout.ap().opt()])
                    # previous step's logits fill the collective dead window
                    if k > 0:
                        emit_logits(steps[k - 1])
                    # ACT-paced fat dummy matmuls keep the PE HAM clock gate
                    # open across the collective wait (array-busy N=512 work;
                    # chain rooted at num_sb so it cannot be hoisted earlier)
                    wkps_full = PS.tile([128, 512], F32, tag="wkps")
                    nc.scalar.copy(wkA[0:1, 0:208], num_sb[:])
                    for i in range(8):
                        ws_, wd_ = (wkA, wkB) if i % 2 == 0 else (wkB, wkA)
                        nc.scalar.copy(wd_[:], ws_[:])
                        nc.tensor.matmul(wkps_full[:], Wg_ctx1[:, 0:128],
                                         wd_[:, 0:512], start=True, stop=True,
                                         skip_group_check=True)
                    nc.sync.dma_start(gather[:], cout[:])

                if "post" in skip:
                    continue
                if fp8ctx:
                    # reduce the 8 gathered [1,208] rows AND transpose to
                    # partition columns in 3 matmuls with a ones selector
                    cu_full = PS.tile([128, 512], F32, tag="cu")
                    g_ = gather[0:8, :]
                    nc.tensor.matmul(cu_full[:, 0:1], g_[:, 1:129], sel8[:],
                                     start=True, stop=True)
                    nc.tensor.matmul(cu_full[0:72, 1:2], g_[:, 129:201],
                                     sel8[:], start=True, stop=True)
                    nc.tensor.matmul(cu_full[:, 2:3],
                                     g_[:, 0:1].to_broadcast((8, 128)),
                                     sel8[:], start=True, stop=True)
                else:
                    # reduce over ranks AND transpose to columns: even rows
                    # carry cols 0:208, odd rows cols 208:416; 0/1 masks
                    # select them into the same PSUM columns
                    KR = 2 if exchange == "none" else 16
                    sE, sO = selEO[0:KR, 0:1], selEO[0:KR, 1:2]
                    cu_full = PS.tile([128, 512], F32, tag="cu")
                    g_ = gather[0:KR, :]
                    nc.tensor.matmul(cu_full[:, 0:1], g_[:, 1:129], sE,
                                     start=True, stop=False)
                    nc.tensor.matmul(cu_full[:, 0:1], g_[:, 209:337], sO,
                                     start=False, stop=True)
                    nc.tensor.matmul(cu_full[0:72, 1:2], g_[:, 129:201], sE,
                                     start=True, stop=False)
                    nc.tensor.matmul(cu_full[0:72, 1:2], g_[:, 337:409], sO,
                                     start=False, stop=True)
                    # den reduced AND broadcast to all 128 partitions in one
                    # matmul (stationary free-dim stride 0 replicates it)
                    nc.tensor.matmul(cu_full[:, 2:3],
                                     g_[:, 0:1].to_broadcast((KR, 128)), sE,
                                     start=True, stop=False)
                    nc.tensor.matmul(cu_full[:, 2:3],
                                     g_[:, 208:209].to_broadcast((KR, 128)),
                                     sO, start=False, stop=True)
                nc.vector.reciprocal(rd[:], cu_full[:, 2:3])
                nc.scalar.activation(ctx_sb[:, 0:1], cu_full[:, 0:1], Copy,
                                     scale=rd[:, 0:1])
                nc.scalar.activation(ctx_sb[0:72, 1:2], cu_full[0:72, 1:2],
                                     Copy, scale=rd[0:72, 0:1])

            # close the gates accumulation with the ctx contributions
            # (all ctx_a matmuls first: they only depend on the first scaled
            # column, so the queue never stalls on the second)
            for g in range(4 if "gates" not in skip else 0):
                gs = slice(100 * g, 100 * (g + 1))
                nc.tensor.matmul(gates_ps[:, g:g + 1], Wg_ctx1[:, gs],
                                 ctx_sb[:, 0:1], start=False, stop=False,
                                 skip_group_check=True)
            for g in range(4 if "gates" not in skip else 0):
                gs = slice(100 * g, 100 * (g + 1))
                nc.tensor.matmul(gates_ps[:, g:g + 1], Wg_ctx2[:, gs],
                                 ctx_sb[0:72, 1:2], start=False, stop=True,
                                 skip_group_check=True)

            # LSTM elementwise
            t_all = W.tile([100, 4], F32, tag="tall")
            if "lstm" in skip:
                continue
            nc.scalar.activation(t_all[:], gates_ps[:], Tanh)
            sig = W.tile([100, 3], F32, tag="sig")
            nc.vector.tensor_scalar(sig[:], t_all[:, 0:3], 1.0, 0.5, ADD, MULT)
            tmp1 = W.tile([100, 1], F32, tag="tmp1")
            tmp2 = W.tile([100, 1], F32, tag="tmp2")
            nc.vector.tensor_tensor(tmp1[:], sig[:, 1:2], c_sb[:], op=MULT)
            nc.vector.tensor_tensor(tmp2[:], sig[:, 0:1], t_all[:, 3:4],
                                    op=MULT)
            nc.vector.tensor_tensor(c_sb[:], tmp1[:], tmp2[:], op=ADD)
            tanh_c = W.tile([100, 1], F32, tag="tanhc")
            nc.scalar.activation(tanh_c[:], c_sb[:], Tanh)
            nc.vector.tensor_tensor(h_aug[0:100, 0:1], sig[:, 2:3], tanh_c[:],
                                    op=MULT)
            if exchange != "collective" and "logits" not in skip:
                emit_logits(t)

        if exchange == "collective":
            emit_logits(steps[-1])
        nc.sync.dma_start(d_S[:], Sbuf[:])
        nc.sync.dma_start(d_sel[:], selbuf[:])

    # cross-core sem waits are invisible to the single-core tile scheduler
    # (it would deadlock); attach them to the scheduled instructions now —
    # compile()'s generate_event_semaphores legalizes multi-wait instructions
    for inst, sem, val in ext_waits:
        inst.wait_op(sem, val, "sem-ge", check=False)

    return nc


# =================== host preprocessing ===================

def _lstm_step_np(x, h, c, W_ih, W_hh, b_ih, b_hh):
    gates = W_ih @ x + b_ih + W_hh @ h + b_hh
    i, f, g, o = np.split(gates, 4)
    sig = lambda v: 1.0 / (1.0 + np.exp(-v))
    c = sig(f) * c + sig(i) * np.tanh(g)
    h = sig(o) * np.tanh(c)
    return h, c


def prep_inputs(inputs, Lc, T):
    """Produce the 8 per-core in_maps from the full problem inputs."""
    im = np.asarray(inputs["input_mat"], np.float32)        # [200, L]
    output_ids = np.asarray(inputs["output_ids"]).astype(np.int64)
    W_ih = np.asarray(inputs["W_ih"], np.float32)
    W_hh = np.asarray(inputs["W_hh"], np.float32)
    b_ih = np.asarray(inputs["b_ih"], np.float32)
    b_hh = np.asarray(inputs["b_hh"], np.float32)
    w1 = np.asarray(inputs["w1"], np.float32)
    w2 = np.asarray(inputs["w2"], np.float32)
    v_w = np.asarray(inputs["v_w"], np.float32)
    lin_w = np.asarray(inputs["lin_w"], np.float32)
    lin_b = np.asarray(inputs["lin_b"], np.float32)
    emb = np.asarray(inputs["emb"], np.float32)
    eos = int(np.asarray(inputs["eos_id"]))

    L = im.shape[1]
    assert Lc * NCORES == L and len(output_ids) == T
    NCH = Lc // 128

    # priming LSTM step on host (exact fp32 math, tiny)
    x0 = np.concatenate([np.zeros(200, np.float32), emb[eos]])
    h0, c0 = _lstm_step_np(x0, np.zeros(100, np.float32),
                           np.zeros(100, np.float32), W_ih, W_hh, b_ih, b_hh)
    h0aug = np.concatenate([h0, [1.0]]).astype(np.float32).reshape(101, 1)

    # gates weights: reorder [i,f,g,o] -> [i,f,o,g], scale i,f,o by 0.5,
    # columns [ctx(200); emb(100); bias(1); h(100)]; transposed for lhsT.
    Wcomb = np.concatenate([W_ih, W_hh], axis=1)            # [400, 400]
    bias = (b_ih + b_hh).astype(np.float32)                 # [400]
    order = np.concatenate([np.arange(100), np.arange(100, 200),
                            np.arange(300, 400), np.arange(200, 300)])
    Wr = Wcomb[order]                                       # rows i,f,o,g
    br = bias[order].copy()
    scale = np.ones((400, 1), np.float32); scale[0:300] = 0.5
    Wr = Wr * scale; br = br * scale[:, 0]
    Wg = np.zeros((401, 400), np.float32)
    # Wcomb columns = [x(300) ; h(100)] where x = [ctx(200); emb(100)]
    Wg[0:200] = Wr[:, 0:200].T       # ctx
    Wg[200:300] = Wr[:, 200:300].T   # emb
    Wg[300] = br                     # bias row
    Wg[301:401] = Wr[:, 300:400].T   # h
    # emb sequence (last_emb per step) with bias-1 row
    emb_seq = np.empty((T, EMB), np.float32)
    emb_seq[0] = emb[eos]
    emb_seq[1:] = emb[output_ids[:T - 1]]
    embaug = np.concatenate([emb_seq.T, np.ones((1, T), np.float32)], axis=0)
    # host-precomputed per-step emb+bias gate contributions:
    # gemb[p, 4t+g] = (Wg[200:301].T @ embaug)[100g+p, t]
    GE = Wg[200:301].T @ embaug                              # [400, T]
    gemb = GE.reshape(4, 100, T).transpose(1, 2, 0).reshape(100, 4 * T)
    gemb = np.ascontiguousarray(gemb, np.float32)
    # logits weights
    linTb = np.concatenate([lin_w.T, lin_b.reshape(1, -1)], axis=0)  # [101,128]
    linsel = np.concatenate([lin_w[output_ids[:T]].T,
                             lin_b[output_ids[:T]].reshape(1, -1)], axis=0)

    w1T = w1.T.copy()                 # [200, 100]
    w2T = w2.T.copy()                 # [200, 100]
    vbf = v_w.reshape(ATT, 1).astype(ml_dtypes.bfloat16)

    in_maps = []
    for cidx in range(NCORES):
        sl = slice(cidx * Lc, (cidx + 1) * Lc)
        imc = im[:, sl]                                    # [200, Lc]
        # imTa[q, c*201 + s] = im[s, c*128+q]; col 200 = 1.0
        blocks = imc.T.reshape(NCH, 128, 200)              # [c, q, s]
        PADW = 23 if FP8CTX else 7
        A = np.concatenate(
            [np.ones((NCH, 128, 1), np.float32), blocks,
             np.zeros((NCH, 128, PADW), np.float32)], axis=2)
        imTa = A.transpose(1, 0, 2).reshape(128, NCH * (201 + PADW))
        imTa = imTa.astype(ml_dtypes.float8_e4m3 if FP8CTX
                           else ml_dtypes.bfloat16)
        selEO = np.zeros((16, 2), np.float32)
        selEO[0::2, 0] = 1.0
        selEO[1::2, 1] = 1.0
        in_maps.append({
            "selEO": selEO,
            "imB": np.ascontiguousarray(imc),
            "imTa": imTa,
            "w1T": w1T, "vb": vbf, "Wg": Wg, "w2T": w2T,
            "linTb": linTb.astype(np.float32),
            "linsel": linsel.astype(np.float32),
            "gemb": gemb,
            "id100": np.eye(100, dtype=np.float32),
            "h0aug": h0aug, "c0": c0.reshape(100, 1).astype(np.float32),
        })
    return in_maps


def finish_loss(Sout, selout):
    """loss = sum_t ( log(sum_j exp(logit_j)) - logit_sel )."""
    S = np.asarray(Sout, np.float64).ravel()
    sel = np.asarray(selout, np.float64).ravel()
    return np.float32(np.sum(np.log(S) - sel))


# =================== self-contained runner ===================
LC = 8192
T_STEPS = 258
FP8CTX = True
_CACHE = {}


def _get_compiled():
    if "nc" not in _CACHE:
        nc = build_kernel(LC, T_STEPS, fp8ctx=FP8CTX)
        nc.compile()
        _CACHE["nc"] = nc
    return _CACHE["nc"]


def kernel(**inputs):
    """Full-input AttnLSTM decoder loss on 8 trn2 cores."""
    from concourse import bass_utils
    nc = _get_compiled()
    in_maps = prep_inputs(inputs, LC, T_STEPS)
    res = bass_utils.run_bass_kernel_spmd(nc, in_maps,
                                          core_ids=list(range(NCORES)))
    out = res.results[0]
    return np.asarray(finish_loss(out["Sout"], out["selout"]))

